# revision 6
# baseline (speedup 1.0000x reference)
"""PointNet feature interpolation (3-NN inverse-distance) Trainium2 kernel.

Problem (per batch b of 8, one NeuronCore each):
  xyz1:    [3, N=8192]   source point coords
  xyz2:    [3, S=2048]   query point coords
  points1: [D=256, N]    source features
  out:     [D, S]        interpolated features

Device algorithm per core:
  1. negdist[s, n] = 2*x2_s.x1_n - |x2_s|^2 - |x1_n|^2  (= -squared distance)
     computed as ONE K=30 bf16 matmul: fp32 values are pre-split on the host
     into (hi, lo, qlo) bf16 triples; all significant cross products are
     separate contraction rows (error ~1e-6, same level as an fp32 reference).
  2. per row: top-8 largest negdist (vector.max) + their indices
     (vector.max_index)  ->  3 nearest neighbors.
  3. weights w_k = (1/(d_k+1e-8)) / sum_k
  4. features gathered by indirect DMA from p1t = points1.T [N, D] in DRAM,
     weighted sum, PE-transpose to [D, S] layout, DMA out.
"""

import numpy as np
import ml_dtypes

B, N, S, D = 8, 8192, 2048, 256
P = 128
NCHUNK = S // P      # 16 row-chunks of queries per core
NT = 512             # matmul moving free dim (one PSUM bank)
NNT = N // NT        # 16
K = 30               # contraction rows of the distance matmul

_COMPILED = None


def _build_bass(reps=1):
    import concourse.bass as bass
    import concourse.mybir as mybir
    import concourse.tile as tile
    from concourse import bacc
    from concourse.masks import make_identity

    f32 = mybir.dt.float32
    bf16 = mybir.dt.bfloat16
    u32 = mybir.dt.uint32
    X = mybir.AxisListType.X
    Alu = mybir.AluOpType

    nc = bacc.Bacc(None)
    x2m = nc.dram_tensor("x2m", [K, S], bf16, kind="ExternalInput")
    x1m = nc.dram_tensor("x1m", [K, N], bf16, kind="ExternalInput")
    p1t = nc.dram_tensor("p1t", [N, D], f32, kind="ExternalInput")
    outT = nc.dram_tensor("outT", [D, S], f32, kind="ExternalOutput")

    with tile.TileContext(nc) as tc:
        with (
            tc.tile_pool(name="const", bufs=1) as cpool,
            tc.tile_pool(name="negd", bufs=2) as ndpool,
            tc.tile_pool(name="mm", bufs=4, space="PSUM") as mmpool,
            tc.tile_pool(name="tr", bufs=2, space="PSUM") as trpool,
            tc.tile_pool(name="small", bufs=4) as spool,
            tc.tile_pool(name="gat", bufs=3) as gpool,
        ):
            x2s = cpool.tile([K, S], bf16)
            nc.sync.dma_start(x2s[:], x2m[:])
            x1s = cpool.tile([K, N], bf16)
            nc.sync.dma_start(x1s[:], x1m[:])
            ident = cpool.tile([P, P], f32)
            make_identity(nc, ident[:])

            for ci_rep in range(NCHUNK * reps):
                ci = ci_rep % NCHUNK
                negd = ndpool.tile([P, N], f32)
                for ni in range(NNT):
                    ps = mmpool.tile([P, NT], f32)
                    nc.tensor.matmul(
                        ps[:],
                        lhsT=x2s[:, ci * P:(ci + 1) * P],
                        rhs=x1s[:, ni * NT:(ni + 1) * NT],
                        start=True,
                        stop=True,
                    )
                    nc.scalar.copy(negd[:, ni * NT:(ni + 1) * NT], ps[:])

                vals8 = spool.tile([P, 8], f32)
                idx8 = spool.tile([P, 8], u32)
                nc.vector.max(out=vals8[:], in_=negd[:])
                nc.vector.max_index(out=idx8[:], in_max=vals8[:], in_values=negd[:])

                # weights: d_k = -v_k ; r_k = 1/(d_k + 1e-8) ; w_k = r_k/sum(r)
                d3 = spool.tile([P, 3], f32)
                nc.vector.tensor_scalar(
                    out=d3[:], in0=vals8[:, 0:3], scalar1=-1.0, scalar2=1e-8,
                    op0=Alu.mult, op1=Alu.add,
                )
                r3 = spool.tile([P, 3], f32)
                nc.vector.reciprocal(r3[:], d3[:])
                rsum = spool.tile([P, 1], f32)
                nc.vector.tensor_reduce(rsum[:], r3[:], axis=X, op=Alu.add)
                rsinv = spool.tile([P, 1], f32)
                nc.vector.reciprocal(rsinv[:], rsum[:])
                w3 = spool.tile([P, 3], f32)
                nc.vector.tensor_scalar(
                    out=w3[:], in0=r3[:], scalar1=rsinv[:, 0:1], scalar2=None,
                    op0=Alu.mult,
                )

                # gather 3 neighbor feature rows per query, weighted-sum
                acc = gpool.tile([P, D], f32, tag="acc")
                for k in range(3):
                    g = gpool.tile([P, D], f32, tag="g")
                    nc.gpsimd.indirect_dma_start(
                        out=g[:],
                        out_offset=None,
                        in_=p1t[:],
                        in_offset=bass.IndirectOffsetOnAxis(
                            ap=idx8[:, k:k + 1], axis=0
                        ),
                    )
                    if k == 0:
                        nc.scalar.mul(acc[:], g[:], w3[:, 0:1])
                    else:
                        gm = gpool.tile([P, D], f32, tag="gm")
                        nc.scalar.mul(gm[:], g[:], w3[:, k:k + 1])
                        nc.vector.tensor_add(acc[:], acc[:], gm[:])

                # transpose [128 s, 256 d] -> two [128 d, 128 s] and store
                for dh in range(2):
                    pt = trpool.tile([P, P], f32)
                    nc.tensor.transpose(
                        pt[:], acc[:, dh * P:(dh + 1) * P], ident[:]
                    )
                    ot = gpool.tile([P, P], f32, tag="ot")
                    nc.scalar.copy(ot[:], pt[:])
                    nc.sync.dma_start(
                        outT[dh * P:(dh + 1) * P, ci * P:(ci + 1) * P], ot[:]
                    )

    nc.finalize()
    return nc


def _split3(x):
    """Split fp64 array into 3 bf16 terms h+l+q ~ x (residual ~2^-27|x|)."""
    bf = ml_dtypes.bfloat16
    h = x.astype(bf)
    r = x - h.astype(np.float64)
    l = r.astype(bf)
    r2 = r - l.astype(np.float64)
    q = r2.astype(bf)
    return h, l, q


def _host_matrices(xyz2b, xyz1b):
    """Build the K=30 bf16 contraction matrices for one batch.

    negdist[s, n] = sum_k X2[k, s] * X1[k, n]
                  = 2 * x2_s . x1_n - |x2_s|^2 - |x1_n|^2
    """
    bf = ml_dtypes.bfloat16
    x2 = xyz2b.astype(np.float64)   # [3, S]
    x1 = xyz1b.astype(np.float64)   # [3, N]
    n2 = (x2 * x2).sum(axis=0)      # [S]
    n1 = (x1 * x1).sum(axis=0)      # [N]

    Srows, Nrows = [], []
    for c in range(3):
        h2, l2, q2 = _split3(x2[c])
        h1, l1, q1 = _split3(x1[c])
        th2 = (2.0 * h2.astype(np.float64)).astype(bf)
        tl2 = (2.0 * l2.astype(np.float64)).astype(bf)
        tq2 = (2.0 * q2.astype(np.float64)).astype(bf)
        # products kept: hh hl lh hq qh ll lq ql   (qq dropped)
        for a, b_ in ((th2, h1), (th2, l1), (tl2, h1), (th2, q1),
                      (tq2, h1), (tl2, l1), (tl2, q1), (tq2, l1)):
            Srows.append(a)
            Nrows.append(b_)
    ones_s = np.ones(x2.shape[1], dtype=bf)
    ones_n = np.ones(x1.shape[1], dtype=bf)
    for t in _split3(-n2):
        Srows.append(t)
        Nrows.append(ones_n)
    for t in _split3(-n1):
        Srows.append(ones_s)
        Nrows.append(t)
    X2 = np.stack([np.asarray(r, dtype=bf) for r in Srows])   # [30, S]
    X1 = np.stack([np.asarray(r, dtype=bf) for r in Nrows])   # [30, N]
    return X2, X1


def _prep_inputs(xyz1, xyz2, points1):
    xyz1 = np.asarray(xyz1, dtype=np.float32)
    xyz2 = np.asarray(xyz2, dtype=np.float32)
    points1 = np.asarray(points1, dtype=np.float32)
    in_maps = []
    for b in range(B):
        X2, X1 = _host_matrices(xyz2[b], xyz1[b])
        p1t = np.ascontiguousarray(points1[b].T)  # [N, D]
        in_maps.append({"x2m": X2, "x1m": X1, "p1t": p1t})
    return in_maps


def _get_compiled():
    global _COMPILED
    if _COMPILED is None:
        _COMPILED = _build_bass()
    return _COMPILED


def kernel(xyz1, xyz2, points1):
    from concourse.bass_utils import run_bass_kernel_spmd

    nc = _get_compiled()
    in_maps = _prep_inputs(xyz1, xyz2, points1)
    res = run_bass_kernel_spmd(nc, in_maps, core_ids=list(range(B)))
    return np.stack([r["outT"] for r in res.results]).astype(np.float32)


if __name__ == "__main__":
    rng = np.random.default_rng(0)
    xyz1 = rng.standard_normal((B, 3, N), dtype=np.float32)
    xyz2 = rng.standard_normal((B, 3, S), dtype=np.float32)
    p1 = rng.standard_normal((B, D, N), dtype=np.float32)
    out = kernel(xyz1, xyz2, p1)
    print("out", out.shape, out.dtype)


# revision 9
# speedup vs baseline: 1.8022x; 1.8022x over previous
"""PointNet feature interpolation (3-NN inverse-distance) Trainium2 kernel.

Problem (per batch b of 8, one NeuronCore each):
  xyz1:    [3, N=8192]   source point coords
  xyz2:    [3, S=2048]   query point coords
  points1: [D=256, N]    source features
  out:     [D, S]        interpolated features

Device algorithm per core:
  1. negdist[s, n] = 2*x2_s.x1_n - |x2_s|^2 - |x1_n|^2  (= -squared distance)
     computed as ONE K=30 bf16 matmul: fp32 values are pre-split on the host
     into (hi, lo, qlo) bf16 triples; all significant cross products are
     separate contraction rows (error ~1e-6, same level as an fp32 reference).
  2. per row: top-8 largest negdist (vector.max) + their indices
     (vector.max_index)  ->  3 nearest neighbors.
  3. weights w_k = (1/(d_k+1e-8)) / sum_k
  4. features gathered by indirect DMA from p1t = points1.T [N, D] in DRAM,
     weighted sum, PE-transpose to [D, S] layout, DMA out.
"""

import numpy as np
import ml_dtypes

B, N, S, D = 8, 8192, 2048, 256
P = 128
NCHUNK = S // P      # 16 row-chunks of queries per core
NT = 512             # matmul moving free dim (one PSUM bank)
NNT = N // NT        # 16
K = 30               # contraction rows of the distance matmul

_COMPILED = None


def _build_bass(reps=1, no_gather=False, no_search=False, no_copy=False):
    import concourse.bass as bass
    import concourse.mybir as mybir
    import concourse.tile as tile
    from concourse import bacc
    from concourse.masks import make_identity

    f32 = mybir.dt.float32
    bf16 = mybir.dt.bfloat16
    u32 = mybir.dt.uint32
    X = mybir.AxisListType.X
    Alu = mybir.AluOpType

    nc = bacc.Bacc(None)
    x2m = nc.dram_tensor("x2m", [K, S], bf16, kind="ExternalInput")
    x1m = nc.dram_tensor("x1m", [K, N], bf16, kind="ExternalInput")
    p1t = nc.dram_tensor("p1t", [N, D], f32, kind="ExternalInput")
    outT = nc.dram_tensor("outT", [D, S], f32, kind="ExternalOutput")

    with tile.TileContext(nc) as tc:
        with (
            tc.tile_pool(name="const", bufs=1) as cpool,
            tc.tile_pool(name="negd", bufs=2) as ndpool,
            tc.tile_pool(name="mm", bufs=4, space="PSUM") as mmpool,
            tc.tile_pool(name="tr", bufs=2, space="PSUM") as trpool,
            tc.tile_pool(name="small", bufs=4) as spool,
            tc.tile_pool(name="gat", bufs=3) as gpool,
        ):
            x2s = cpool.tile([K, S], bf16)
            nc.sync.dma_start(x2s[:], x2m[:])
            x1s = cpool.tile([K, N], bf16)
            nc.sync.dma_start(x1s[:], x1m[:])
            ident = cpool.tile([P, P], f32)
            make_identity(nc, ident[:])

            for ci_rep in range(NCHUNK * reps):
                ci = ci_rep % NCHUNK
                negd = ndpool.tile([P, N], f32)
                for ni in range(NNT):
                    ps = mmpool.tile([P, NT], f32)
                    nc.tensor.matmul(
                        ps[:],
                        lhsT=x2s[:, ci * P:(ci + 1) * P],
                        rhs=x1s[:, ni * NT:(ni + 1) * NT],
                        start=True,
                        stop=True,
                    )
                    if no_copy:
                        if ni == 0:
                            nc.scalar.copy(negd[:, 0:NT], ps[:])
                    else:
                        nc.scalar.copy(negd[:, ni * NT:(ni + 1) * NT], ps[:])

                vals8 = spool.tile([P, 8], f32)
                idx8 = spool.tile([P, 8], u32)
                if no_search:
                    nc.vector.memset(vals8[:], -1.0)
                    nc.vector.memset(idx8[:], 7)
                else:
                    nc.vector.max(out=vals8[:], in_=negd[:])
                    nc.vector.max_index(out=idx8[:], in_max=vals8[:],
                                        in_values=negd[:])

                # weights: d_k = -v_k ; r_k = 1/(d_k + 1e-8) ; w_k = r_k/sum(r)
                d3 = spool.tile([P, 3], f32)
                nc.vector.tensor_scalar(
                    out=d3[:], in0=vals8[:, 0:3], scalar1=-1.0, scalar2=1e-8,
                    op0=Alu.mult, op1=Alu.add,
                )
                r3 = spool.tile([P, 3], f32)
                nc.vector.reciprocal(r3[:], d3[:])
                rsum = spool.tile([P, 1], f32)
                nc.vector.tensor_reduce(rsum[:], r3[:], axis=X, op=Alu.add)
                rsinv = spool.tile([P, 1], f32)
                nc.vector.reciprocal(rsinv[:], rsum[:])
                w3 = spool.tile([P, 3], f32)
                nc.vector.tensor_scalar(
                    out=w3[:], in0=r3[:], scalar1=rsinv[:, 0:1], scalar2=None,
                    op0=Alu.mult,
                )

                # gather 3 neighbor feature rows per query, weighted-sum
                acc = gpool.tile([P, D], f32, tag="acc")
                for k in range(3):
                    g = gpool.tile([P, D], f32, tag="g")
                    if no_gather:
                        nc.sync.dma_start(g[:], p1t[k * P:(k + 1) * P, :])
                    else:
                        nc.gpsimd.indirect_dma_start(
                            out=g[:],
                            out_offset=None,
                            in_=p1t[:],
                            in_offset=bass.IndirectOffsetOnAxis(
                                ap=idx8[:, k:k + 1], axis=0
                            ),
                        )
                    if k == 0:
                        nc.scalar.mul(acc[:], g[:], w3[:, 0:1])
                    else:
                        gm = gpool.tile([P, D], f32, tag="gm")
                        nc.scalar.mul(gm[:], g[:], w3[:, k:k + 1])
                        nc.vector.tensor_add(acc[:], acc[:], gm[:])

                # transpose [128 s, 256 d] -> two [128 d, 128 s] and store
                for dh in range(2):
                    pt = trpool.tile([P, P], f32)
                    nc.tensor.transpose(
                        pt[:], acc[:, dh * P:(dh + 1) * P], ident[:]
                    )
                    ot = gpool.tile([P, P], f32, tag="ot")
                    nc.scalar.copy(ot[:], pt[:])
                    nc.sync.dma_start(
                        outT[dh * P:(dh + 1) * P, ci * P:(ci + 1) * P], ot[:]
                    )

    nc.finalize()
    return nc


def _split3(x):
    """Split fp64 array into 3 bf16 terms h+l+q ~ x (residual ~2^-27|x|)."""
    bf = ml_dtypes.bfloat16
    h = x.astype(bf)
    r = x - h.astype(np.float64)
    l = r.astype(bf)
    r2 = r - l.astype(np.float64)
    q = r2.astype(bf)
    return h, l, q


def _host_matrices(xyz2b, xyz1b):
    """Build the K=30 bf16 contraction matrices for one batch.

    negdist[s, n] = sum_k X2[k, s] * X1[k, n]
                  = 2 * x2_s . x1_n - |x2_s|^2 - |x1_n|^2
    """
    bf = ml_dtypes.bfloat16
    x2 = xyz2b.astype(np.float64)   # [3, S]
    x1 = xyz1b.astype(np.float64)   # [3, N]
    n2 = (x2 * x2).sum(axis=0)      # [S]
    n1 = (x1 * x1).sum(axis=0)      # [N]

    Srows, Nrows = [], []
    for c in range(3):
        h2, l2, q2 = _split3(x2[c])
        h1, l1, q1 = _split3(x1[c])
        th2 = (2.0 * h2.astype(np.float64)).astype(bf)
        tl2 = (2.0 * l2.astype(np.float64)).astype(bf)
        tq2 = (2.0 * q2.astype(np.float64)).astype(bf)
        # products kept: hh hl lh hq qh ll lq ql   (qq dropped)
        for a, b_ in ((th2, h1), (th2, l1), (tl2, h1), (th2, q1),
                      (tq2, h1), (tl2, l1), (tl2, q1), (tq2, l1)):
            Srows.append(a)
            Nrows.append(b_)
    ones_s = np.ones(x2.shape[1], dtype=bf)
    ones_n = np.ones(x1.shape[1], dtype=bf)
    for t in _split3(-n2):
        Srows.append(t)
        Nrows.append(ones_n)
    for t in _split3(-n1):
        Srows.append(ones_s)
        Nrows.append(t)
    X2 = np.stack([np.asarray(r, dtype=bf) for r in Srows])   # [30, S]
    X1 = np.stack([np.asarray(r, dtype=bf) for r in Nrows])   # [30, N]
    return X2, X1


def _prep_inputs(xyz1, xyz2, points1):
    xyz1 = np.asarray(xyz1, dtype=np.float32)
    xyz2 = np.asarray(xyz2, dtype=np.float32)
    points1 = np.asarray(points1, dtype=np.float32)
    in_maps = []
    for b in range(B):
        X2, X1 = _host_matrices(xyz2[b], xyz1[b])
        p1t = np.ascontiguousarray(points1[b].T)  # [N, D]
        in_maps.append({"x2m": X2, "x1m": X1, "p1t": p1t})
    return in_maps


def _get_compiled():
    global _COMPILED
    if _COMPILED is None:
        _COMPILED = _build_bass()
    return _COMPILED


def kernel(xyz1, xyz2, points1):
    from concourse.bass_utils import run_bass_kernel_spmd

    nc = _get_compiled()
    in_maps = _prep_inputs(xyz1, xyz2, points1)
    res = run_bass_kernel_spmd(nc, in_maps, core_ids=list(range(B)))
    return np.stack([r["outT"] for r in res.results]).astype(np.float32)


if __name__ == "__main__":
    rng = np.random.default_rng(0)
    xyz1 = rng.standard_normal((B, 3, N), dtype=np.float32)
    xyz2 = rng.standard_normal((B, 3, S), dtype=np.float32)
    p1 = rng.standard_normal((B, D, N), dtype=np.float32)
    out = kernel(xyz1, xyz2, p1)
    print("out", out.shape, out.dtype)
